# revision 13
# baseline (speedup 1.0000x reference)
"""Trainium2 Bass kernel for the GNO (Galerkin-type linear attention) model.

Reference computation per batch element b (N=4096 tokens, d=64):
    h = x @ lift_w + lift_b
    for each of 4 layers:
        q = h@q_w+q_b ; k = h@k_w+k_b ; v = h@v_w+v_b
        kern     = (q @ k^T) / sqrt(d)          # [N, N], no softmax!
        integral = (kern @ v) / N               # [N, d]
        h        = gelu(h@blk_w+blk_b + integral)
    out = h @ proj_w + proj_b

Math restructure (all biases folded via an augmented ones row/column):
    (q k^T) v == q (k^T v)                 (no softmax)
    k^T v     == kvk^T G kvv  with  G = H_aug^T H_aug   ([65,65] Gram)
    q (k^T v) == H_aug ( C G kvv )  with  C = (qw_aug*s) kvk^T  (host-precomputed)
    layer     == gelu( H_aug @ (blkw_aug + C G kvv) )
so each layer is: 32 PE transposes of H (to get token-major tiles), a 32-step
Gram accumulation, a tiny 3-matmul chain, and one [65,64]-weight update matmul.
All matmul operands are bf16 (PE runs 4x faster than fp32); accumulation stays
fp32 in PSUM.  rel err vs fp32 reference ~2.4e-3 (tolerance 2e-2).

Sharding: batch is 2 -> data-parallel on 2 NeuronCores, one batch element per
core, everything resident in SBUF.  Sequence-sharding wider would need a
per-layer AllReduce of the [64,64] moment matrix whose latency floor exceeds
the whole per-layer compute, so it loses.
"""

import os
import sys

for _p in ("/opt/trn_rl_repo", "/root/.axon_site/_ro/trn_rl_repo"):
    if os.path.isdir(_p) and _p not in sys.path:
        sys.path.append(_p)

import numpy as np

N = 4096          # tokens per batch element (64*64)
D = 64            # hidden
DA = D + 1        # hidden + ones row
L = 4             # layers
B = 2             # batch / cores used
SCALE = (1.0 / np.sqrt(np.float32(D))) / np.float32(N)

# wpack column layout: [liftw 65 | I66 66 | kvv 4*64 | CT 4*65 | blkw 4*64 | projw 1]
# I66 = [I65 | 0]: the zero pad column keeps bf16 PSUM transpose outputs
# 4-byte aligned (66*2B slots) while writing every PSUM cell.
DP = DA + 1  # padded tile stride (66)
OFF_LIFT = 0
OFF_I = 65
OFF_KVV = OFF_I + DP
OFF_CT = OFF_KVV + 4 * D
OFF_BLK = OFF_CT + 4 * DA
OFF_PROJ = OFF_BLK + 4 * D
WCOLS = OFF_PROJ + 1

_CACHE = {}


def _build_nc():
    """Build + compile the per-core Bass program (identical on both cores)."""
    import concourse.bass as bass
    import concourse.tile as tile
    from concourse import bacc, mybir

    f32 = mybir.dt.float32
    bf16 = mybir.dt.bfloat16
    ts = bass.ts
    GELU = mybir.ActivationFunctionType.Gelu

    nc = bacc.Bacc("TRN2", target_bir_lowering=False, debug=False, num_devices=B)

    xt_d = nc.dram_tensor("xt", [4, N], bf16, kind="ExternalInput")
    wp_d = nc.dram_tensor("wpack", [DA, WCOLS], bf16, kind="ExternalInput")
    y_d = nc.dram_tensor("y", [128, 32], f32, kind="ExternalOutput")

    PS = bass.MemorySpace.PSUM

    with tile.TileContext(nc) as tc:
        with (
            tc.tile_pool(name="consts", bufs=1) as consts,
            tc.tile_pool(name="hbuf", bufs=1) as hbuf,
            tc.tile_pool(name="htp", bufs=2) as htp,
            tc.tile_pool(name="small", bufs=2) as small,
            tc.tile_pool(name="gtp", bufs=2) as gtp,
            tc.tile_pool(name="ps_tp", bufs=2, space=PS) as ps_tp,
            tc.tile_pool(name="ps_sm", bufs=3, space=PS) as ps_sm,
            tc.tile_pool(name="ps_up", bufs=2, space=PS) as ps_up,
        ):
            # ---- load inputs into SBUF (parallel DMA queues) -------------
            xt = consts.tile([4, N], bf16, tag="xt")
            nc.sync.dma_start(xt[:, 0 : N // 2], xt_d.ap()[:, 0 : N // 2])
            nc.scalar.dma_start(xt[:, N // 2 :], xt_d.ap()[:, N // 2 :])
            wp = consts.tile([DA, WCOLS], bf16, tag="wp")
            nc.gpsimd.dma_start(wp[:], wp_d.ap())

            I65 = wp[:, OFF_I : OFF_I + DA]       # square identity
            I66 = wp[:, OFF_I : OFF_I + DP]       # identity + zero pad col
            projw = wp[:, OFF_PROJ : OFF_PROJ + 1]

            # two ping-pong H_aug buffers, [65, 4096] each; ones row for H1
            # comes from xt's ones row (H0's from the lift matmul).
            H0 = hbuf.tile([DA, N], bf16, tag="h0")
            H1 = hbuf.tile([DA, N], bf16, tag="h1")
            nc.sync.dma_start(H1[D : D + 1, :], xt_d.ap()[3:4, :])

            # ---- lift: H0 = lift_aug^T @ xt  ([65, 4096]) ----------------
            # casts alternate vector/scalar so neither engine serializes it.
            for c in range(8):
                lf_ps = ps_up.tile([DA, 512], f32, tag="up")
                nc.tensor.matmul(lf_ps[:], wp[0:4, OFF_LIFT : OFF_LIFT + DA],
                                 xt[:, ts(c, 512)], start=True, stop=True)
                if c % 2 == 0:
                    nc.vector.tensor_copy(H0[:, ts(c, 512)], lf_ps[:])
                else:
                    nc.scalar.copy(H0[:, ts(c, 512)], lf_ps[:])

            # ---- layers --------------------------------------------------
            for l in range(L):
                cur = H0 if l % 2 == 0 else H1
                nxt = H1 if l % 2 == 0 else H0
                kvv_l = wp[:, OFF_KVV + l * D : OFF_KVV + (l + 1) * D]
                ct_l = wp[:, OFF_CT + l * DA : OFF_CT + (l + 1) * DA]
                blk_l = wp[:, OFF_BLK + l * D : OFF_BLK + (l + 1) * D]

                # W_upd accumulator: start with blkw (via I65), finish after
                # the chain below adds C @ G @ kvv.
                w_ps = ps_sm.tile([DA, D], f32, tag="sm")
                nc.tensor.matmul(w_ps[:], I65, blk_l, start=True, stop=False)

                g_ps = ps_sm.tile([DA, DA], f32, tag="sm")
                HT = htp.tile([128, 32 * DP], bf16, tag="ht")

                # software-pipelined: transpose group g, copy it to SBUF,
                # then Gram-accumulate group g-1 (whose copy has landed).
                def g_group(g):
                    for k in range(4):
                        t = 4 * g + k
                        nc.tensor.matmul(
                            g_ps[:], HT[:, t * DP : t * DP + DA],
                            HT[:, t * DP : t * DP + DA],
                            start=(t == 0), stop=(t == 31))

                for g in range(8):
                    tp_ps = ps_tp.tile([128, 4 * DP], bf16, tag="tp")
                    for k in range(4):
                        t = 4 * g + k
                        nc.tensor.transpose(tp_ps[:, k * DP : (k + 1) * DP],
                                            cur[:, ts(t, 128)], I66)
                    nc.vector.tensor_copy(
                        HT[:, g * 4 * DP : (g + 1) * 4 * DP], tp_ps[:])
                    if g > 0:
                        g_group(g - 1)
                g_group(7)

                # chain: wupd = blkw + C @ G @ kvv (casts on scalar: it is
                # idle here and sits closer to PSUM)
                g_sb = small.tile([DA, DA], bf16, tag="gsb")
                nc.scalar.copy(g_sb[:], g_ps[:])
                m1_ps = ps_sm.tile([DA, D], f32, tag="sm")
                nc.tensor.matmul(m1_ps[:], g_sb[:], kvv_l, start=True, stop=True)
                m1_sb = small.tile([DA, D], bf16, tag="m1")
                nc.scalar.copy(m1_sb[:], m1_ps[:])
                nc.tensor.matmul(w_ps[:], ct_l, m1_sb[:], start=False, stop=True)
                wupd = small.tile([DA, D], bf16, tag="wupd")
                nc.scalar.copy(wupd[:], w_ps[:])

                # update: h' = gelu(H_aug^T @ W_upd).  Chunks are processed
                # in pairs: the two matmuls land in PSUM partitions 0-63 /
                # 64-127 (PE column tiling), so one [128,512] gelu covers
                # both (full scalar-engine width).  The upper half is moved
                # back to partitions 0-63 by an SBUF->SBUF DMA.
                if l == L - 1:
                    y_ps = ps_sm.tile([128, 32], f32, tag="sm")
                for p in range(4):
                    up_ps = ps_up.tile([128, 512], f32, tag="up")
                    nc.tensor.matmul(up_ps[0:D, :], wupd[:],
                                     cur[:, ts(2 * p, 512)],
                                     start=True, stop=True)
                    nc.tensor.matmul(up_ps[64:128, :], wupd[:],
                                     cur[:, ts(2 * p + 1, 512)],
                                     start=True, stop=True)
                    gt = gtp.tile([128, 512], bf16, tag="gt")
                    nc.scalar.activation(gt[:], up_ps[:], GELU)
                    nc.vector.tensor_copy(nxt[0:D, ts(2 * p, 512)],
                                          gt[0:D, :])
                    nc.gpsimd.dma_start(nxt[0:D, ts(2 * p + 1, 512)],
                                        gt[64:128, :])
                    if l == L - 1:
                        # proj (token-major, y_ps[q, t] = y[128*t + q]),
                        # interleaved so the tail is just a cast + DMA
                        nxt_chunks = [2 * p, 2 * p + 1]
                        for cc in nxt_chunks:
                            for tt in range(4 * cc, 4 * cc + 4):
                                nc.tensor.matmul(
                                    y_ps[:, tt : tt + 1],
                                    nxt[:, ts(tt, 128)], projw,
                                    start=True, stop=True)

            out_sb = consts.tile([128, 32], f32, tag="out")
            nc.vector.tensor_copy(out_sb[:], y_ps[:])
            nc.sync.dma_start(y_d.ap(), out_sb[:])

    nc.compile()
    return nc


def _prep_inputs(x, lift_w, lift_b, blk_w, blk_b, q_w, q_b, k_w, k_b, v_w,
                 v_b, proj_w, proj_b):
    """Host-side weight packing (tiny [64,64] reshuffles, negligible cost)."""
    import ml_dtypes
    bf16 = ml_dtypes.bfloat16
    f = lambda a: np.asarray(a, dtype=np.float32)
    x = f(x)
    lift_w, lift_b = f(lift_w), f(lift_b)
    blk_w, blk_b = f(blk_w), f(blk_b)
    q_w, q_b, k_w, k_b, v_w, v_b = f(q_w), f(q_b), f(k_w), f(k_b), f(v_w), f(v_b)
    proj_w, proj_b = f(proj_w), f(proj_b)

    wpack = np.zeros((DA, WCOLS), np.float32)
    # lift_aug in rows 0..3: [lift_w; lift_b] with a 1 that emits H0's ones row
    wpack[0:3, OFF_LIFT : OFF_LIFT + D] = lift_w
    wpack[3, OFF_LIFT : OFF_LIFT + D] = lift_b
    wpack[3, OFF_LIFT + D] = 1.0
    wpack[:, OFF_I : OFF_I + DA] = np.eye(DA, dtype=np.float32)  # pad col 0
    for l in range(L):
        kvk = np.vstack([k_w[l], k_b[l][None]])            # [65, 64]
        kvv = np.vstack([v_w[l], v_b[l][None]])            # [65, 64]
        qts = (np.vstack([q_w[l], q_b[l][None]]) * SCALE).T  # [64, 65]
        ct = kvk @ qts                                     # [65, 65] = C^T
        wpack[:, OFF_KVV + l * D : OFF_KVV + (l + 1) * D] = kvv
        wpack[:, OFF_CT + l * DA : OFF_CT + (l + 1) * DA] = ct
        wpack[:, OFF_BLK + l * D : OFF_BLK + (l + 1) * D] = \
            np.vstack([blk_w[l], blk_b[l][None]])
    wpack[:, OFF_PROJ] = np.concatenate([proj_w[:, 0], proj_b])
    wpack = wpack.astype(bf16)

    in_maps = []
    for b in range(B):
        xt = np.concatenate([x[b].reshape(N, 3).T,
                             np.ones((1, N), np.float32)], axis=0).astype(bf16)
        in_maps.append({"xt": np.ascontiguousarray(xt), "wpack": wpack})
    return in_maps, x.shape


def _get_runner():
    """Compile once, return a fn(in_maps) -> list[{name: np.ndarray}]."""
    if "runner" in _CACHE:
        return _CACHE["runner"]

    import jax
    from jax.sharding import Mesh, PartitionSpec
    try:
        from jax.experimental.shard_map import shard_map
    except ImportError:  # newer jax
        from jax.sharding import shard_map
    from concourse import mybir
    from concourse.bass2jax import (_bass_exec_p, install_neuronx_cc_hook,
                                    partition_id_tensor)

    nc = _build_nc()
    install_neuronx_cc_hook()

    partition_name = (nc.partition_id_tensor.name
                      if nc.partition_id_tensor else None)
    in_names, out_names, out_avals, zero_outs = [], [], [], []
    for alloc in nc.m.functions[0].allocations:
        if not isinstance(alloc, mybir.MemoryLocationSet):
            continue
        name = alloc.memorylocations[0].name
        if alloc.kind == "ExternalInput":
            if name != partition_name:
                in_names.append(name)
        elif alloc.kind == "ExternalOutput":
            shape = tuple(alloc.tensor_shape)
            dtype = mybir.dt.np(alloc.dtype)
            out_names.append(name)
            out_avals.append(jax.core.ShapedArray(shape, dtype))
            zero_outs.append(np.zeros(shape, dtype))
    n_params = len(in_names)
    n_outs = len(out_avals)
    all_in_names = in_names + out_names + ([partition_name] if partition_name else [])
    donate = tuple(range(n_params, n_params + n_outs))

    def _body(*args):
        operands = list(args)
        if partition_name is not None:
            operands.append(partition_id_tensor())
        return tuple(_bass_exec_p.bind(
            *operands, out_avals=tuple(out_avals), in_names=tuple(all_in_names),
            out_names=tuple(out_names), lowering_input_output_aliases=(),
            sim_require_finite=True, sim_require_nnan=True, nc=nc))

    devices = jax.devices()[:B]
    mesh = Mesh(np.asarray(devices), ("core",))
    sharded = jax.jit(
        shard_map(_body, mesh=mesh,
                  in_specs=(PartitionSpec("core"),) * (n_params + n_outs),
                  out_specs=(PartitionSpec("core"),) * n_outs,
                  check_rep=False),
        donate_argnums=donate, keep_unused=True)

    def run(in_maps):
        per_core = [[np.asarray(m[name]) for name in in_names] for m in in_maps]
        concat_in = [np.concatenate([per_core[c][i] for c in range(B)], axis=0)
                     for i in range(n_params)]
        big_zeros = [np.concatenate([z] * B, axis=0) for z in zero_outs]
        outs = jax.block_until_ready(sharded(*concat_in, *big_zeros))
        results = []
        for c in range(B):
            r = {}
            for i, name in enumerate(out_names):
                rows = out_avals[i].shape[0]
                r[name] = np.asarray(outs[i][c * rows : (c + 1) * rows])
            results.append(r)
        return results

    _CACHE["runner"] = run
    return run


def kernel(**inputs) -> np.ndarray:
    in_maps, x_shape = _prep_inputs(**inputs)
    run = _get_runner()
    results = run(in_maps)
    # y_core [128, 32]: element (p, t) = y[128*t + p] -> transpose to linear
    out = np.stack([results[b]["y"].T.reshape(x_shape[1], x_shape[2], 1)
                    for b in range(B)])
    return out.astype(np.float32)


# revision 15
# speedup vs baseline: 1.1368x; 1.1368x over previous
"""Trainium2 Bass kernel for the GNO (Galerkin-type linear attention) model.

Reference computation per batch element b (N=4096 tokens, d=64):
    h = x @ lift_w + lift_b
    for each of 4 layers:
        q = h@q_w+q_b ; k = h@k_w+k_b ; v = h@v_w+v_b
        kern     = (q @ k^T) / sqrt(d)          # [N, N], no softmax!
        integral = (kern @ v) / N               # [N, d]
        h        = gelu(h@blk_w+blk_b + integral)
    out = h @ proj_w + proj_b

Math restructure (all biases folded via an augmented ones row/column):
    (q k^T) v == q (k^T v)                 (no softmax)
    k^T v     == kvk^T G kvv  with  G = H_aug^T H_aug   ([65,65] Gram)
    q (k^T v) == H_aug ( C G kvv )  with  C = (qw_aug*s) kvk^T  (host-precomputed)
    layer     == gelu( H_aug @ (blkw_aug + C G kvv) )
so each layer is: 32 PE transposes of H (to get token-major tiles), a 32-step
Gram accumulation, a tiny 3-matmul chain, and one [65,64]-weight update matmul.
All matmul operands are bf16 (PE runs 4x faster than fp32); accumulation stays
fp32 in PSUM.  rel err vs fp32 reference ~2.4e-3 (tolerance 2e-2).

Sharding: batch is 2 -> data-parallel on 2 NeuronCores, one batch element per
core, everything resident in SBUF.  Sequence-sharding wider would need a
per-layer AllReduce of the [64,64] moment matrix whose latency floor exceeds
the whole per-layer compute, so it loses.
"""

import os
import sys

for _p in ("/opt/trn_rl_repo", "/root/.axon_site/_ro/trn_rl_repo"):
    if os.path.isdir(_p) and _p not in sys.path:
        sys.path.append(_p)

import numpy as np

N = 4096          # tokens per batch element (64*64)
D = 64            # hidden
DA = D + 1        # hidden + ones row
L = 4             # layers
B = 2             # batch / cores used
SCALE = (1.0 / np.sqrt(np.float32(D))) / np.float32(N)

# wpack column layout: [liftw 65 | I66 66 | kvv 4*64 | CT 4*65 | blkw 4*64 | projw 1]
# I66 = [I65 | 0]: the zero pad column keeps bf16 PSUM transpose outputs
# 4-byte aligned (66*2B slots) while writing every PSUM cell.
DP = DA + 1  # padded tile stride (66)
OFF_LIFT = 0
OFF_I = 65
OFF_KVV = OFF_I + DP
OFF_CT = OFF_KVV + 4 * D
OFF_BLK = OFF_CT + 4 * DA
OFF_PROJ = OFF_BLK + 4 * D
WCOLS = OFF_PROJ + 1

_CACHE = {}


def _build_nc():
    """Build + compile the per-core Bass program (identical on both cores)."""
    import concourse.bass as bass
    import concourse.tile as tile
    from concourse import bacc, mybir

    f32 = mybir.dt.float32
    bf16 = mybir.dt.bfloat16
    ts = bass.ts
    GELU = mybir.ActivationFunctionType.Gelu

    nc = bacc.Bacc("TRN2", target_bir_lowering=False, debug=False, num_devices=B)

    xt_d = nc.dram_tensor("xt", [4, N], bf16, kind="ExternalInput")
    wp_d = nc.dram_tensor("wpack", [DA, WCOLS], bf16, kind="ExternalInput")
    y_d = nc.dram_tensor("y", [128, 32], f32, kind="ExternalOutput")

    PS = bass.MemorySpace.PSUM

    with tile.TileContext(nc) as tc:
        with (
            tc.tile_pool(name="consts", bufs=1) as consts,
            tc.tile_pool(name="hbuf", bufs=1) as hbuf,
            tc.tile_pool(name="htp", bufs=2) as htp,
            tc.tile_pool(name="small", bufs=2) as small,
            tc.tile_pool(name="gtp", bufs=4) as gtp,
            tc.tile_pool(name="ps_tp", bufs=2, space=PS) as ps_tp,
            tc.tile_pool(name="ps_sm", bufs=3, space=PS) as ps_sm,
            tc.tile_pool(name="ps_up", bufs=3, space=PS) as ps_up,
        ):
            # ---- load inputs into SBUF (parallel DMA queues) -------------
            xt = consts.tile([4, N], bf16, tag="xt")
            nc.sync.dma_start(xt[:], xt_d.ap())
            wp = consts.tile([DA, WCOLS], bf16, tag="wp")
            nc.gpsimd.dma_start(wp[:], wp_d.ap())

            # warm up the PE clock (HAM un-throttles 1.2->2.4 GHz after
            # ~3.4us of sustained activity) with junk matmuls on memset
            # data while the input DMAs are still in flight.
            junk = consts.tile([DA, 512], bf16, tag="junk")
            nc.vector.memset(junk[:], 0.0)
            jk_ps = ps_up.tile([DA, 512], f32, tag="up")
            for _ in range(8):
                nc.tensor.matmul(jk_ps[:], junk[:, 0:DA], junk[:],
                                 start=True, stop=True)

            I65 = wp[:, OFF_I : OFF_I + DA]       # square identity
            I66 = wp[:, OFF_I : OFF_I + DP]       # identity + zero pad col
            projw = wp[:, OFF_PROJ : OFF_PROJ + 1]

            # two ping-pong H_aug buffers, [65, 4096] each; ones row for H1
            # comes from xt's ones row (H0's from the lift matmul).
            H0 = hbuf.tile([DA, N], bf16, tag="h0")
            H1 = hbuf.tile([DA, N], bf16, tag="h1")
            nc.gpsimd.dma_start(H1[D : D + 1, :], xt_d.ap()[3:4, :])

            # ---- lift: H0 = lift_aug^T @ xt  ([65, 4096]) ----------------
            # casts alternate vector/scalar so neither engine serializes it.
            for c in range(8):
                lf_ps = ps_up.tile([DA, 512], f32, tag="up")
                nc.tensor.matmul(lf_ps[:], wp[0:4, OFF_LIFT : OFF_LIFT + DA],
                                 xt[:, ts(c, 512)], start=True, stop=True)
                if c % 2 == 0:
                    nc.vector.tensor_copy(H0[:, ts(c, 512)], lf_ps[:])
                else:
                    nc.scalar.copy(H0[:, ts(c, 512)], lf_ps[:])

            # ---- layers --------------------------------------------------
            for l in range(L):
                cur = H0 if l % 2 == 0 else H1
                nxt = H1 if l % 2 == 0 else H0
                kvv_l = wp[:, OFF_KVV + l * D : OFF_KVV + (l + 1) * D]
                ct_l = wp[:, OFF_CT + l * DA : OFF_CT + (l + 1) * DA]
                blk_l = wp[:, OFF_BLK + l * D : OFF_BLK + (l + 1) * D]

                # W_upd accumulator: start with blkw (via I65), finish after
                # the chain below adds C @ G @ kvv.
                w_ps = ps_sm.tile([DA, D], f32, tag="sm")
                nc.tensor.matmul(w_ps[:], I65, blk_l, start=True, stop=False)

                g_ps = ps_sm.tile([DA, DA], f32, tag="sm")
                HT = htp.tile([128, 32 * DP], bf16, tag="ht")

                # software-pipelined: transpose group g, copy it to SBUF,
                # then Gram-accumulate group g-1 (whose copy has landed).
                def g_group(g):
                    for k in range(4):
                        t = 4 * g + k
                        nc.tensor.matmul(
                            g_ps[:], HT[:, t * DP : t * DP + DA],
                            HT[:, t * DP : t * DP + DA],
                            start=(t == 0), stop=(t == 31))

                for g in range(8):
                    tp_ps = ps_tp.tile([128, 4 * DP], bf16, tag="tp")
                    for k in range(4):
                        t = 4 * g + k
                        nc.tensor.transpose(tp_ps[:, k * DP : (k + 1) * DP],
                                            cur[:, ts(t, 128)], I66)
                    nc.vector.tensor_copy(
                        HT[:, g * 4 * DP : (g + 1) * 4 * DP], tp_ps[:])
                    if g > 0:
                        g_group(g - 1)
                g_group(7)

                # chain: wupd = blkw + C @ G @ kvv (casts on scalar: it is
                # idle here and sits closer to PSUM)
                g_sb = small.tile([DA, DA], bf16, tag="gsb")
                nc.vector.tensor_copy(g_sb[:], g_ps[:])
                m1_ps = ps_sm.tile([DA, D], f32, tag="sm")
                nc.tensor.matmul(m1_ps[:], g_sb[:], kvv_l, start=True, stop=True)
                m1_sb = small.tile([DA, D], bf16, tag="m1")
                nc.vector.tensor_copy(m1_sb[:], m1_ps[:])
                nc.tensor.matmul(w_ps[:], ct_l, m1_sb[:], start=False, stop=True)
                wupd = small.tile([DA, D], bf16, tag="wupd")
                nc.vector.tensor_copy(wupd[:], w_ps[:])

                # update: h' = gelu(H_aug^T @ W_upd).  Chunks are processed
                # in pairs: the two matmuls land in PSUM partitions 0-63 /
                # 64-127 (PE column tiling), so one [128,512] gelu covers
                # both (full scalar-engine width).  The upper half is moved
                # back to partitions 0-63 by an SBUF->SBUF DMA.
                if l == L - 1:
                    y_ps = ps_sm.tile([128, 32], f32, tag="sm")
                # pair chunks (p, p+4): the DMA-shifted upper half lands in
                # chunks 4-7, which the next layer's transposes need LAST,
                # hiding the SBUF->SBUF DMA latency.
                for p in range(4):
                    up_ps = ps_up.tile([128, 512], f32, tag="up")
                    nc.tensor.matmul(up_ps[0:D, :], wupd[:],
                                     cur[:, ts(p, 512)],
                                     start=True, stop=True)
                    nc.tensor.matmul(up_ps[64:128, :], wupd[:],
                                     cur[:, ts(p + 4, 512)],
                                     start=True, stop=True)
                    gt = gtp.tile([128, 512], bf16, tag="gt")
                    nc.scalar.activation(gt[:], up_ps[:], GELU)
                    nc.vector.tensor_copy(nxt[0:D, ts(p, 512)],
                                          gt[0:D, :])
                    nc.gpsimd.dma_start(nxt[0:D, ts(p + 4, 512)],
                                        gt[64:128, :])
                    if l == L - 1:
                        # proj (token-major, y_ps[q, t] = y[128*t + q]),
                        # interleaved so the tail is just a cast + DMA
                        for cc in (p, p + 4):
                            for tt in range(4 * cc, 4 * cc + 4):
                                nc.tensor.matmul(
                                    y_ps[:, tt : tt + 1],
                                    nxt[:, ts(tt, 128)], projw,
                                    start=True, stop=True)

            # split the output drain so the first half's cast+DMA overlaps
            # the last update pair's proj matmuls
            out_sb = consts.tile([128, 32], f32, tag="out")
            nc.vector.tensor_copy(out_sb[:, 0:16], y_ps[:, 0:16])
            nc.sync.dma_start(y_d.ap()[:, 0:16], out_sb[:, 0:16])
            nc.vector.tensor_copy(out_sb[:, 16:32], y_ps[:, 16:32])
            nc.sync.dma_start(y_d.ap()[:, 16:32], out_sb[:, 16:32])

    nc.compile()
    return nc


def _prep_inputs(x, lift_w, lift_b, blk_w, blk_b, q_w, q_b, k_w, k_b, v_w,
                 v_b, proj_w, proj_b):
    """Host-side weight packing (tiny [64,64] reshuffles, negligible cost)."""
    import ml_dtypes
    bf16 = ml_dtypes.bfloat16
    f = lambda a: np.asarray(a, dtype=np.float32)
    x = f(x)
    lift_w, lift_b = f(lift_w), f(lift_b)
    blk_w, blk_b = f(blk_w), f(blk_b)
    q_w, q_b, k_w, k_b, v_w, v_b = f(q_w), f(q_b), f(k_w), f(k_b), f(v_w), f(v_b)
    proj_w, proj_b = f(proj_w), f(proj_b)

    wpack = np.zeros((DA, WCOLS), np.float32)
    # lift_aug in rows 0..3: [lift_w; lift_b] with a 1 that emits H0's ones row
    wpack[0:3, OFF_LIFT : OFF_LIFT + D] = lift_w
    wpack[3, OFF_LIFT : OFF_LIFT + D] = lift_b
    wpack[3, OFF_LIFT + D] = 1.0
    wpack[:, OFF_I : OFF_I + DA] = np.eye(DA, dtype=np.float32)  # pad col 0
    for l in range(L):
        kvk = np.vstack([k_w[l], k_b[l][None]])            # [65, 64]
        kvv = np.vstack([v_w[l], v_b[l][None]])            # [65, 64]
        qts = (np.vstack([q_w[l], q_b[l][None]]) * SCALE).T  # [64, 65]
        ct = kvk @ qts                                     # [65, 65] = C^T
        wpack[:, OFF_KVV + l * D : OFF_KVV + (l + 1) * D] = kvv
        wpack[:, OFF_CT + l * DA : OFF_CT + (l + 1) * DA] = ct
        wpack[:, OFF_BLK + l * D : OFF_BLK + (l + 1) * D] = \
            np.vstack([blk_w[l], blk_b[l][None]])
    wpack[:, OFF_PROJ] = np.concatenate([proj_w[:, 0], proj_b])
    wpack = wpack.astype(bf16)

    in_maps = []
    for b in range(B):
        xt = np.concatenate([x[b].reshape(N, 3).T,
                             np.ones((1, N), np.float32)], axis=0).astype(bf16)
        in_maps.append({"xt": np.ascontiguousarray(xt), "wpack": wpack})
    return in_maps, x.shape


def _get_runner():
    """Compile once, return a fn(in_maps) -> list[{name: np.ndarray}]."""
    if "runner" in _CACHE:
        return _CACHE["runner"]

    import jax
    from jax.sharding import Mesh, PartitionSpec
    try:
        from jax.experimental.shard_map import shard_map
    except ImportError:  # newer jax
        from jax.sharding import shard_map
    from concourse import mybir
    from concourse.bass2jax import (_bass_exec_p, install_neuronx_cc_hook,
                                    partition_id_tensor)

    nc = _build_nc()
    install_neuronx_cc_hook()

    partition_name = (nc.partition_id_tensor.name
                      if nc.partition_id_tensor else None)
    in_names, out_names, out_avals, zero_outs = [], [], [], []
    for alloc in nc.m.functions[0].allocations:
        if not isinstance(alloc, mybir.MemoryLocationSet):
            continue
        name = alloc.memorylocations[0].name
        if alloc.kind == "ExternalInput":
            if name != partition_name:
                in_names.append(name)
        elif alloc.kind == "ExternalOutput":
            shape = tuple(alloc.tensor_shape)
            dtype = mybir.dt.np(alloc.dtype)
            out_names.append(name)
            out_avals.append(jax.core.ShapedArray(shape, dtype))
            zero_outs.append(np.zeros(shape, dtype))
    n_params = len(in_names)
    n_outs = len(out_avals)
    all_in_names = in_names + out_names + ([partition_name] if partition_name else [])
    donate = tuple(range(n_params, n_params + n_outs))

    def _body(*args):
        operands = list(args)
        if partition_name is not None:
            operands.append(partition_id_tensor())
        return tuple(_bass_exec_p.bind(
            *operands, out_avals=tuple(out_avals), in_names=tuple(all_in_names),
            out_names=tuple(out_names), lowering_input_output_aliases=(),
            sim_require_finite=True, sim_require_nnan=True, nc=nc))

    devices = jax.devices()[:B]
    mesh = Mesh(np.asarray(devices), ("core",))
    sharded = jax.jit(
        shard_map(_body, mesh=mesh,
                  in_specs=(PartitionSpec("core"),) * (n_params + n_outs),
                  out_specs=(PartitionSpec("core"),) * n_outs,
                  check_rep=False),
        donate_argnums=donate, keep_unused=True)

    def run(in_maps):
        per_core = [[np.asarray(m[name]) for name in in_names] for m in in_maps]
        concat_in = [np.concatenate([per_core[c][i] for c in range(B)], axis=0)
                     for i in range(n_params)]
        big_zeros = [np.concatenate([z] * B, axis=0) for z in zero_outs]
        outs = jax.block_until_ready(sharded(*concat_in, *big_zeros))
        results = []
        for c in range(B):
            r = {}
            for i, name in enumerate(out_names):
                rows = out_avals[i].shape[0]
                r[name] = np.asarray(outs[i][c * rows : (c + 1) * rows])
            results.append(r)
        return results

    _CACHE["runner"] = run
    return run


def kernel(**inputs) -> np.ndarray:
    in_maps, x_shape = _prep_inputs(**inputs)
    run = _get_runner()
    results = run(in_maps)
    # y_core [128, 32]: element (p, t) = y[128*t + p] -> transpose to linear
    out = np.stack([results[b]["y"].T.reshape(x_shape[1], x_shape[2], 1)
                    for b in range(B)])
    return out.astype(np.float32)
